# revision 4
# baseline (speedup 1.0000x reference)
"""Trainium2 Bass kernel v2 for nn_Boundary_Enchance (dense_cnn).

Data parallel: core i processes batch image i.  The heavy matmuls (fuse 1x1
conv and the 3x3 conv) run as fp8e4 DoubleRow matmuls: two K=128 k-tiles per
instruction at 0.5 cycles per output column -- 4x the bf16 rate.  End-to-end
fp8 error measured at ~3e-3 (budget 2e-2).

Per-core structure:
  prebarrier - per 8-row tile t (stride 6): fuse_box = relu(1x1conv(y)+b) as
    one DoubleRow matmul (y rows 0..3 = k-tile 0, rows 4..7 = k-tile 1, bias
    via a preset ones partition).  Evacuation (rotating DVE/Act/Pool) writes
    the persistent fp8 F region and per-tile row sums (accum_out).
  SE chain - row sums -> selection matmul -> gap -> MLP -> sigmoid ->
    LBM = PSB*se + LM (data-dependent mask+boundary lhsT, M=12).
  main loop - fronts run 4+ strips ahead of 4-strip tail blocks:
    front: conv3x3 over concat(F, x) as 3 DoubleRow matmuls (k-tile 0 = F
      region, k-tile 1 = x ring; dx taps via guard-column shifted views, all
      full-width);  fcc = relu(conv+b) evacuated bf16 into the persistent
      fccbig region (y rides at partitions 97..126 for the boundary head).
    tail block (4 strips): mask+boundary logits as bf16 matmuls into [12,
      1024] PSUM pairs, one sigmoid per pair, batched add+min -> scale s;
      out = cv_w*s + cv_b via a replication matmul (M=96) + one affine
      tensor_scalar during the PSUM->SBUF evacuation; bf16 output DMA.
"""

import numpy as np
import ml_dtypes

BF16 = ml_dtypes.bfloat16
F8 = ml_dtypes.float8_e4m3

H = 512
W = 512
SB = 6
NT = (H + SB - 1) // SB          # 86 strips
NPIX = float(H * W)
PITCH = W + 2                    # F / x slot pitch (guard cols at 0, 513)
XSLOTS = 16
YSLOTS = 16
XMID = 43                        # x-ring sits between F(42) and F(43)
W8C = 3 * 256 + 6 * 192          # w8 cols: LF_F|LF|LF_L + WCF0..2 + WCX0..2
WBC = 96 + 96 + 128 + 96 + 12 + 12  # SEL W1L W2R LCREP LM PSB
NBLK = (NT + 3) // 4             # 4-strip tail blocks (last partial)
FLAG = 4

_cache = {}


# ----------------------------------------------------------------------------
# host-side weight layout builders
# ----------------------------------------------------------------------------

def _fcol(s):
    return (s if s < XMID else s + XSLOTS) * PITCH


def _lf(fuse_w, fuse_b, variant):
    """[128, 2, 128] fuse lhsT.  partition p=r'*5+c (r'<4), ones at p=20.
    k-tile k covers tile rows r=4k+r'.  out col m=r*16+oc."""
    out = np.zeros((128, 2, 128), np.float32)
    fw = fuse_w[:, :, 0, 0]                       # [16 oc, 5 ic]
    for k in range(2):
        for rp in range(4):
            r = 4 * k + rp
            for c in range(5):
                out[rp * 5 + c, k, r * 16:r * 16 + 16] = fw[:, c]
    out[20, 0, :] = np.tile(fuse_b, 8)            # bias on ones partition
    if variant == "first":
        out[:, :, 0:16] = 0.0
    elif variant == "last":
        out[:, :, 48:128] = 0.0
    return out


def _wc(fc_w, forder):
    """[3][128, 2, 96] conv lhsT per dx.  p=r*16+c; k-tiles = (F half, x
    half) if forder else (x half, F half); out col m=i*16+oc, taps 0..2."""
    out = np.zeros((3, 128, 2, 96), np.float32)
    kf, kx = (0, 1) if forder else (1, 0)
    for dx in range(3):
        for i in range(SB):
            for ky in range(3):
                r = i + ky
                out[dx, r * 16:r * 16 + 16, kf, i * 16:i * 16 + 16] = \
                    fc_w[:, 16:32, ky, dx].T
                out[dx, r * 16:r * 16 + 16, kx, i * 16:i * 16 + 16] = \
                    fc_w[:, 0:16, ky, dx].T
    return out


def _lm(fm_w, fm_b, bd_b):
    out = np.zeros((128, 12), np.float32)
    d = fm_w[1, :, 0, 0] - fm_w[0, :, 0, 0]
    for i in range(SB):
        out[i * 16:i * 16 + 16, i] = d
    out[96, 0:SB] = fm_b[1] - fm_b[0]
    out[96, 6:6 + SB] = bd_b[1] - bd_b[0]
    return out


def _psb(bd_w):
    out = np.zeros((128, 12), np.float32)
    d = bd_w[1, :, 0, 0] - bd_w[0, :, 0, 0]
    for r in range(SB):
        out[97 + r * 5:97 + r * 5 + 5, 6 + r] = d
    return out


def _sel():
    out = np.zeros((128, 96), np.float32)
    for r in range(1, 7):
        for fc in range(16):
            out[r * 16 + fc, fc] = 1.0 / NPIX
    return out


def _w1l(se_w1):
    out = np.zeros((128, 96), np.float32)
    out[:16, :16] = se_w1.T
    return out


def _w2r(se_w2):
    out = np.zeros((128, 128), np.float32)
    for r in range(SB):
        out[:16, 97 + r * 5:97 + r * 5 + 5] = se_w2.T
    return out


def _lcrep():
    out = np.zeros((128, 96), np.float32)
    for i in range(SB):
        out[i, i * 16:i * 16 + 16] = 1.0
        out[6 + i, i * 16:i * 16 + 16] = 1.0
    return out


def _pack_w8(fuse_w, fuse_b, fc_w):
    blocks = [_lf(fuse_w, fuse_b, "first").reshape(128, 256),
              _lf(fuse_w, fuse_b, "mid").reshape(128, 256),
              _lf(fuse_w, fuse_b, "last").reshape(128, 256)]
    wcf = _wc(fc_w, True)
    blocks += [wcf[dx].reshape(128, 192) for dx in range(3)]
    wcx = _wc(fc_w, False)
    blocks += [wcx[dx].reshape(128, 192) for dx in range(3)]
    return np.concatenate(blocks, axis=1).astype(F8)


def _pack_wb(se_w1, se_w2, fm_w, fm_b, bd_w, bd_b):
    blocks = [_sel(), _w1l(se_w1), _w2r(se_w2), _lcrep(),
              _lm(fm_w, fm_b, bd_b), _psb(bd_w)]
    return np.concatenate(blocks, axis=1).astype(BF16)


# ----------------------------------------------------------------------------
# bass graph
# ----------------------------------------------------------------------------

def _build():
    import concourse.bass as bass
    import concourse.bacc as bacc
    import concourse.tile as tile
    from concourse import mybir

    f32 = mybir.dt.float32
    bf16 = mybir.dt.bfloat16
    fp8 = mybir.dt.float8e4
    AF = mybir.ActivationFunctionType
    ALU = mybir.AluOpType
    DR = mybir.MatmulPerfMode.DoubleRow

    def _mid(v, stride, n):
        """Insert a middle dim [stride, n] into a [P, C] view -> [P, n, C]."""
        return bass.AP(v.tensor, v.offset,
                       [list(v.ap[0]), [stride, n], list(v.ap[1])])

    nc = bacc.Bacc("TRN2", target_bir_lowering=False)
    xp_ext = nc.declare_dram_parameter("xp", [NT, 128, W], fp8, isOutput=False)
    yh_ext = nc.declare_dram_parameter("yhp", [NT, 20, 1024], fp8,
                                       isOutput=False)
    yc_ext = nc.declare_dram_parameter("ycp", [NT, 30, W], bf16,
                                       isOutput=False)
    w8_ext = nc.declare_dram_parameter("w8", [128, W8C], fp8, isOutput=False)
    wb_ext = nc.declare_dram_parameter("wb", [128, WBC], bf16, isOutput=False)
    fcb_ext = nc.declare_dram_parameter("fcb", [96, 1], f32, isOutput=False)
    cvw_ext = nc.declare_dram_parameter("cvw", [96, 4], f32, isOutput=False)
    c8_ext = nc.declare_dram_parameter("c8", [108, 1024], fp8, isOutput=False)
    cb_ext = nc.declare_dram_parameter("cb", [2, NT * W], bf16,
                                   isOutput=False)
    out_ext = nc.declare_dram_parameter("out", [NT, 96, W], bf16,
                                        isOutput=True)

    with tile.TileContext(nc) as tc:
        with tc.tile_pool(name="singles", bufs=1) as singles:
            # ---------------- persistent SBUF ----------------
            big = singles.tile([128, (NT + XSLOTS) * PITCH], fp8, tag="big")
            fccbig = singles.tile([128, NT * W], bf16, tag="fccbig")
            yht = singles.tile([128, YSLOTS * 1024], fp8, tag="yht")
            w8 = singles.tile([128, W8C], fp8, tag="w8")
            wb = singles.tile([128, WBC], bf16, tag="wb")
            fcb = singles.tile([96, 1], f32, tag="fcb")
            cvw = singles.tile([96, 4], f32, tag="cvw")
            Ra = singles.tile([128, NT], f32, tag="Ra")
            sigring = singles.tile([128, 4 * 2048], bf16, tag="sigring")
            rept = singles.tile([96, 4 * 2048], bf16, tag="rept")
            LBM = singles.tile([128, 12], bf16, tag="lbm")

            nc.sync.dma_start(out=w8[:, :], in_=w8_ext[:, :])
            nc.sync.dma_start(out=wb[:, :], in_=wb_ext[:, :])
            nc.sync.dma_start(out=fcb[:, :], in_=fcb_ext[:, :])
            nc.sync.dma_start(out=cvw[:, :], in_=cvw_ext[:, :])

            LF_F = _mid(w8[:, 0:128], 128, 2)
            LF = _mid(w8[:, 256:384], 128, 2)
            LF_L = _mid(w8[:, 512:640], 128, 2)
            WCF = [_mid(w8[:, 768 + dx * 192:768 + dx * 192 + 96], 96, 2)
                   for dx in range(3)]
            WCX = [_mid(w8[:, 1344 + dx * 192:1344 + dx * 192 + 96], 96, 2)
                   for dx in range(3)]
            SEL = wb[:, 0:96]
            W1L = wb[:, 96:192]
            W2R = wb[:, 192:320]
            LCREP = wb[:, 320:416]
            LM = wb[:, 416:428]
            PSB = wb[:, 428:440]

            # guard cols of F region and x ring (cols 0 and 513 of each slot)
            gv = big[:, 513:515]
            nc.vector.memset(
                bass.AP(gv.tensor, gv.offset,
                        [list(gv.ap[0]), [PITCH, NT + XSLOTS - 1], [1, 2]]),
                0.0)
            nc.vector.memset(big[:, 0:1], 0.0)
            nc.vector.memset(big[:, (NT + XSLOTS) * PITCH - 1:
                                 (NT + XSLOTS) * PITCH], 0.0)

            # yht slot 0..3 presets up front; the other 12 slots and all
            # phase-2 presets are issued behind the first input batches
            c8v = c8_ext[:, :]
            nc.sync.dma_start(
                out=yht[20:128, 0:4 * 1024],
                in_=bass.AP(c8v.tensor, c8v.offset,
                            [list(c8v.ap[0]), [0, 4], list(c8v.ap[1])]))

            def late_presets(stage):
                if stage < 3:
                    # yht slots 4..7 / 8..11 / 12..15
                    lo = 4 + 4 * stage
                    nc.sync.dma_start(
                        out=yht[20:128, lo * 1024:(lo + 4) * 1024],
                        in_=bass.AP(c8v.tensor, c8v.offset,
                                    [list(c8v.ap[0]), [0, 4],
                                     list(c8v.ap[1])]))
                else:
                    nc.sync.dma_start(out=fccbig[96:97, :], in_=cb_ext[0:1, :])
                    nc.sync.dma_start(out=fccbig[127:128, :],
                                      in_=cb_ext[1:2, :])
                    z4 = cb_ext[1:2, 0:8192]
                    nc.sync.dma_start(
                        out=sigring[12:128, :],
                        in_=bass.AP(z4.tensor, z4.offset,
                                    [list(z4.ap[0]), [0, 116],
                                     list(z4.ap[1])]))

            # ========== phase 1: fuse + conv fronts, then SE ============
            ctx2 = tc.tile_pool(name="psC", bufs=2, space="PSUM")
            psC = ctx2.__enter__()
            with tc.tile_pool(name="psF", bufs=2, space="PSUM") as psF:
                fhold = {}

                def fuse(t):
                    if t % 4 == 0:
                        n = min(4, NT - t)
                        k0 = t % YSLOTS
                        nc.sync.dma_start(
                            out=yht[0:20, k0 * 1024:(k0 + n) * 1024],
                            in_=yh_ext[t:t + n, :, :].rearrange(
                                "s p j -> p s j"))
                        if t <= 12:
                            late_presets(t // 4)
                    yslot = yht[:, (t % YSLOTS) * 1024:
                                (t % YSLOTS) * 1024 + 512]
                    lf = LF_F if t == 0 else (LF_L if t == NT - 1 else LF)
                    if t % 2 == 0:
                        fhold["t"] = psF.tile([128, 1024], f32, tag="fuse",
                                              name="fusepair")
                    fps = fhold["t"]
                    h = (t % 2) * W
                    nc.tensor.matmul(fps[:, h:h + W], lhsT=lf,
                                     rhs=_mid(yslot, 512, 2),
                                     start=True, stop=True, perf_mode=DR)
                    if t % 2 == 1:
                        c0 = _fcol(t - 1) + 1
                        d = _fcol(t) - _fcol(t - 1)
                        fv = big[:, c0:c0 + W]
                        fdst = bass.AP(fv.tensor, fv.offset,
                                       [list(fv.ap[0]), [d, 2], [1, W]])
                        nc.vector.tensor_scalar(
                            out=fdst, in0=fps[:, :], scalar1=0.0, scalar2=0.0,
                            op0=ALU.max, op1=ALU.add,
                            accum_out=Ra[:, t // 2:t // 2 + 1])

                def front(s):
                    if s % 4 == 0:
                        n = min(4, NT - s)
                        j0 = s % XSLOTS
                        xc0 = (XMID + j0) * PITCH
                        dst = big[:, xc0 + 1:xc0 + 1 + W]
                        nc.sync.dma_start(
                            out=bass.AP(dst.tensor, dst.offset,
                                        [list(dst.ap[0]), [PITCH, n], [1, W]]),
                            in_=xp_ext[s:s + n, :, :].rearrange(
                                "s p j -> p s j"))
                        if s % 8 == 0:
                            n2 = min(8, NT - s)
                            nc.sync.dma_start(
                                out=fccbig[97:127, s * W:(s + n2) * W],
                                in_=yc_ext[s:s + n2, :, :].rearrange(
                                    "s p j -> p s j"))
                    fcol = _fcol(s)
                    xcol = (XMID + s % XSLOTS) * PITCH
                    base, delta = ((fcol, xcol - fcol) if s < XMID
                                   else (xcol, fcol - xcol))
                    wcs = WCF if s < XMID else WCX
                    if s % 2 == 0:
                        fhold["s"] = psC.tile([96, 1024], f32, tag="conv",
                                              name="convpair")
                    cps = fhold["s"]
                    hh = (s % 2) * W
                    for dx in range(3):
                        v = big[:, base + dx:base + dx + W]
                        nc.tensor.matmul(cps[:, hh:hh + W], lhsT=wcs[dx],
                                         rhs=_mid(v, delta, 2),
                                         start=(dx == 0), stop=(dx == 2),
                                         perf_mode=DR)
                    if s % 2 == 1:
                        fdst = fccbig[0:96, (s - 1) * W:(s + 1) * W]
                        nc.scalar.activation(
                            out=fdst, in_=cps[:, :], func=AF.Relu,
                            bias=fcb[:, :])

                for i in range(NT + FLAG):
                    if i < NT:
                        fuse(i)
                    if i >= FLAG:
                        front(i - FLAG)

                # ================= SE chain ==============================
                NP2 = NT // 2
                Rbf = singles.tile([128, NP2], bf16, tag="Rbf")
                nc.vector.tensor_copy(out=Rbf[:, :], in_=Ra[:, 0:NP2])
                gps = psF.tile([96, NP2], f32, tag="fuse")
                nc.tensor.matmul(gps[:, :], lhsT=SEL, rhs=Rbf[:, :],
                                 start=True, stop=True)
                gap_f = singles.tile([96, 1], f32, tag="gapf")
                nc.vector.reduce_sum(out=gap_f[:, :], in_=gps[:, :],
                                     axis=mybir.AxisListType.X)
                gap_bf = singles.tile([128, 1], bf16, tag="gap")
                nc.vector.memset(gap_bf[:, :], 0.0)
                nc.vector.tensor_copy(out=gap_bf[0:96, :], in_=gap_f[:, :])
                hps = psF.tile([96, 1], f32, tag="fuse")
                nc.tensor.matmul(hps[:, :], lhsT=W1L, rhs=gap_bf[:, :],
                                 start=True, stop=True)
                h_bf = singles.tile([128, 1], bf16, tag="hbf")
                nc.vector.memset(h_bf[:, :], 0.0)
                nc.scalar.activation(out=h_bf[0:96, :], in_=hps[:, :],
                                     func=AF.Relu)
                sps = psF.tile([128, 1], f32, tag="fuse")
                nc.tensor.matmul(sps[:, :], lhsT=W2R, rhs=h_bf[:, :],
                                 start=True, stop=True)
                se_bc = singles.tile([128, 1], f32, tag="sebc")
                nc.scalar.activation(out=se_bc[:, :], in_=sps[:, :],
                                     func=AF.Sigmoid)
                nc.vector.scalar_tensor_tensor(
                    out=LBM[:, :], in0=PSB, scalar=se_bc[:, :], in1=LM,
                    op0=ALU.mult, op1=ALU.add)

            ctx2.__exit__(None, None, None)
            # ========== phase 2: tails ==================================
            with (
                tc.tile_pool(name="psM", bufs=2, space="PSUM") as psM,
                tc.tile_pool(name="psV", bufs=2, space="PSUM") as psV,
            ):
                def tail_block(b):
                    u0 = b * 4
                    ns = min(4, NT - u0)
                    w4 = ns * W
                    sslot = (b % 4) * 2048
                    npair = (ns + 1) // 2
                    for p in range(npair):
                        u = u0 + 2 * p
                        np_ = min(2, NT - u)
                        mps = psM.tile([12, 1024], f32, tag="mb")
                        for q in range(np_):
                            nc.tensor.matmul(
                                mps[:, q * W:(q + 1) * W], lhsT=LBM[:, :],
                                rhs=fccbig[:, (u + q) * W:(u + q + 1) * W],
                                start=True, stop=True)
                        nc.scalar.activation(
                            out=sigring[0:12, sslot + p * 1024:
                                        sslot + p * 1024 + np_ * W],
                            in_=mps[:, 0:np_ * W], func=AF.Sigmoid)
                    # v = sigma_m + sigma_b, replicated x16 via one matmul;
                    # evacuation folds min(v,1)*cv_w; then +cv_b in 4x mode.
                    rp = rept[:, (b % 4) * 2048:(b % 4) * 2048 + w4]
                    aform = (b % 4 == 0)
                    for q in range(ns):
                        vps = psV.tile([96, W], f32, tag="cv")
                        nc.tensor.matmul(
                            vps[:, :], lhsT=LCREP,
                            rhs=sigring[:, sslot + q * W:sslot + (q + 1) * W],
                            start=True, stop=True)
                        og = rept[:, (b % 4) * 2048 + q * W:
                                  (b % 4) * 2048 + (q + 1) * W]
                        if aform:
                            # r = relu(1 - v) on Act; min(v,1) = 1 - r
                            nc.scalar.activation(
                                out=og, in_=vps[:, :], func=AF.Relu,
                                scale=-1.0, bias=1.0)
                        else:
                            nc.vector.tensor_scalar(
                                out=og, in0=vps[:, :], scalar1=1.0,
                                scalar2=cvw[:, 0:1], op0=ALU.min,
                                op1=ALU.mult)
                    if aform:
                        # out = -w*r + (w+b)
                        nc.vector.tensor_scalar(
                            out=rp, in0=rp, scalar1=cvw[:, 2:3],
                            scalar2=cvw[:, 3:4], op0=ALU.mult, op1=ALU.add)
                    elif b >= NBLK - 3:
                        nc.vector.tensor_scalar(
                            out=rp, in0=rp, scalar1=cvw[:, 1:2],
                            scalar2=None, op0=ALU.add)
                    else:
                        nc.gpsimd.tensor_scalar(
                            out=rp, in0=rp, scalar1=cvw[:, 1:2],
                            scalar2=None, op0=ALU.add)
                    nc.sync.dma_start(
                        out=out_ext[u0:u0 + ns, :, :].rearrange(
                            "s p j -> p s j"),
                        in_=rp)

                for b in range(NBLK):
                    tail_block(b)
    nc.compile()
    return nc


# ----------------------------------------------------------------------------
# entry point
# ----------------------------------------------------------------------------

LAST_RESULT = None


def prepare(x, y, fuse_w, fuse_b, se_w1, se_w2, bd_w, bd_b,
            fc_w, fc_b, fm_w, fm_b, cv_w, cv_b):
    if "nc" not in _cache:
        _cache["nc"] = _build()
    nc = _cache["nc"]

    g = {k: np.asarray(v, np.float32) for k, v in dict(
        fuse_w=fuse_w, fuse_b=fuse_b, se_w1=se_w1, se_w2=se_w2, bd_w=bd_w,
        bd_b=bd_b, fc_w=fc_w, fc_b=fc_b, fm_w=fm_w, fm_b=fm_b, cv_w=cv_w,
        cv_b=cv_b).items()}
    w8 = _pack_w8(g["fuse_w"], g["fuse_b"], g["fc_w"])
    wb = _pack_wb(g["se_w1"], g["se_w2"], g["fm_w"], g["fm_b"],
                  g["bd_w"], g["bd_b"])
    fcb = np.zeros((96, 1), np.float32)
    cvw = np.zeros((96, 4), np.float32)
    for i in range(SB):
        fcb[i * 16:(i + 1) * 16, 0] = g["fc_b"]
        cvw[i * 16:(i + 1) * 16, 0] = g["cv_w"][:, 0, 0, 0]
        cvw[i * 16:(i + 1) * 16, 1] = g["cv_b"]
    cvw[:, 2] = -cvw[:, 0]
    cvw[:, 3] = cvw[:, 0] + cvw[:, 1]

    c8 = np.zeros((108, 1024), F8)       # partitions 20..127 of a yht slot
    c8[0, 0:512] = 1.0                   # ones partition (20), ktile0 only
    cb = np.zeros((2, NT * W), BF16)
    cb[0, :] = 1.0                       # ones row; row 1 zeros

    xb = np.asarray(x, np.float32)
    yb = np.asarray(y, np.float32)
    B = xb.shape[0]

    xpad = np.zeros((B, 16, 6 * NT + 8, W), np.float32)
    xpad[:, :, 1:H + 1, :] = xb
    ridx = 6 * np.arange(NT)[:, None] + np.arange(8)[None, :]
    xp = xpad[:, :, ridx, :].transpose(0, 2, 3, 1, 4) \
        .reshape(B, NT, 128, W).astype(F8)

    ypad = np.zeros((B, 5, 6 * NT + 8, W), np.float32)
    ypad[:, :, 1:H + 1, :] = yb
    # yhp[b,t, r'*5+c, k*512+n] = ypad[b, c, 6t+4k+r', n]
    yidx = (6 * np.arange(NT)[:, None, None]
            + 4 * np.arange(2)[None, :, None]
            + np.arange(4)[None, None, :])
    yhp = ypad[:, :, yidx, :]                     # [B, 5, NT, 2, 4, W]
    yhp = yhp.transpose(0, 2, 4, 1, 3, 5).reshape(B, NT, 20, 2 * W).astype(F8)

    cidx = 6 * np.arange(NT)[:, None] + 1 + np.arange(6)[None, :]
    ycp = ypad[:, :, cidx, :].transpose(0, 2, 3, 1, 4) \
        .reshape(B, NT, 30, W).astype(BF16)

    in_maps = [
        {"xp": np.ascontiguousarray(xp[i]),
         "yhp": np.ascontiguousarray(yhp[i]),
         "ycp": np.ascontiguousarray(ycp[i]),
         "w8": w8, "wb": wb, "fcb": fcb, "cvw": cvw, "c8": c8, "cb": cb}
        for i in range(B)
    ]
    return nc, in_maps


def kernel(x, y, fuse_w, fuse_b, se_w1, se_w2, bd_w, bd_b,
           fc_w, fc_b, fm_w, fm_b, cv_w, cv_b):
    global LAST_RESULT
    from concourse.bass_utils import run_bass_kernel_spmd

    nc, in_maps = prepare(x, y, fuse_w, fuse_b, se_w1, se_w2, bd_w, bd_b,
                          fc_w, fc_b, fm_w, fm_b, cv_w, cv_b)
    res = run_bass_kernel_spmd(nc, in_maps, core_ids=list(range(8)))
    LAST_RESULT = res
    outs = []
    for i in range(len(in_maps)):
        ot = np.asarray(res.results[i]["out"], np.float32)
        full = ot.reshape(NT, SB, 16, W).transpose(2, 0, 1, 3) \
                 .reshape(16, NT * SB, W)[:, :H, :]
        outs.append(full)
    return np.stack(outs)


# revision 5
# speedup vs baseline: 1.0559x; 1.0559x over previous
"""Trainium2 Bass kernel v2 for nn_Boundary_Enchance (dense_cnn).

Data parallel: core i processes batch image i.  The heavy matmuls (fuse 1x1
conv and the 3x3 conv) run as fp8e4 DoubleRow matmuls: two K=128 k-tiles per
instruction at 0.5 cycles per output column -- 4x the bf16 rate.  End-to-end
fp8 error measured at ~3e-3 (budget 2e-2).

Per-core structure:
  prebarrier - per 8-row tile t (stride 6): fuse_box = relu(1x1conv(y)+b) as
    one DoubleRow matmul (y rows 0..3 = k-tile 0, rows 4..7 = k-tile 1, bias
    via a preset ones partition).  Evacuation (rotating DVE/Act/Pool) writes
    the persistent fp8 F region and per-tile row sums (accum_out).
  SE chain - row sums -> selection matmul -> gap -> MLP -> sigmoid ->
    LBM = PSB*se + LM (data-dependent mask+boundary lhsT, M=12).
  main loop - fronts run 4+ strips ahead of 4-strip tail blocks:
    front: conv3x3 over concat(F, x) as 3 DoubleRow matmuls (k-tile 0 = F
      region, k-tile 1 = x ring; dx taps via guard-column shifted views, all
      full-width);  fcc = relu(conv+b) evacuated bf16 into the persistent
      fccbig region (y rides at partitions 97..126 for the boundary head).
    tail block (4 strips): mask+boundary logits as bf16 matmuls into [12,
      1024] PSUM pairs, one sigmoid per pair, batched add+min -> scale s;
      out = cv_w*s + cv_b via a replication matmul (M=96) + one affine
      tensor_scalar during the PSUM->SBUF evacuation; bf16 output DMA.
"""

import numpy as np
import ml_dtypes

BF16 = ml_dtypes.bfloat16
F8 = ml_dtypes.float8_e4m3

H = 512
W = 512
SB = 6
NT = (H + SB - 1) // SB          # 86 strips
NPIX = float(H * W)
PITCH = W + 2                    # F / x slot pitch (guard cols at 0, 513)
XSLOTS = 16
YSLOTS = 16
XMID = 43                        # x-ring sits between F(42) and F(43)
W8C = 3 * 256 + 6 * 192          # w8 cols: LF_F|LF|LF_L + WCF0..2 + WCX0..2
WBC = 96 + 96 + 128 + 96 + 12 + 12  # SEL W1L W2R LCREP LM PSB
NBLK = (NT + 3) // 4             # 4-strip tail blocks (last partial)
FLAG = 4

_cache = {}


# ----------------------------------------------------------------------------
# host-side weight layout builders
# ----------------------------------------------------------------------------

def _fcol(s):
    return (s if s < XMID else s + XSLOTS) * PITCH


def _lf(fuse_w, fuse_b, variant):
    """[128, 2, 128] fuse lhsT.  partition p=r'*5+c (r'<4), ones at p=20.
    k-tile k covers tile rows r=4k+r'.  out col m=r*16+oc."""
    out = np.zeros((128, 2, 128), np.float32)
    fw = fuse_w[:, :, 0, 0]                       # [16 oc, 5 ic]
    for k in range(2):
        for rp in range(4):
            r = 4 * k + rp
            for c in range(5):
                out[rp * 5 + c, k, r * 16:r * 16 + 16] = fw[:, c]
    out[20, 0, :] = np.tile(fuse_b, 8)            # bias on ones partition
    if variant == "first":
        out[:, :, 0:16] = 0.0
    elif variant == "last":
        out[:, :, 48:128] = 0.0
    return out


def _wc(fc_w, forder):
    """[3][128, 2, 96] conv lhsT per dx.  p=r*16+c; k-tiles = (F half, x
    half) if forder else (x half, F half); out col m=i*16+oc, taps 0..2."""
    out = np.zeros((3, 128, 2, 96), np.float32)
    kf, kx = (0, 1) if forder else (1, 0)
    for dx in range(3):
        for i in range(SB):
            for ky in range(3):
                r = i + ky
                out[dx, r * 16:r * 16 + 16, kf, i * 16:i * 16 + 16] = \
                    fc_w[:, 16:32, ky, dx].T
                out[dx, r * 16:r * 16 + 16, kx, i * 16:i * 16 + 16] = \
                    fc_w[:, 0:16, ky, dx].T
    return out


def _lm(fm_w, fm_b, bd_b):
    out = np.zeros((128, 12), np.float32)
    d = fm_w[1, :, 0, 0] - fm_w[0, :, 0, 0]
    for i in range(SB):
        out[i * 16:i * 16 + 16, i] = d
    out[96, 0:SB] = fm_b[1] - fm_b[0]
    out[96, 6:6 + SB] = bd_b[1] - bd_b[0]
    return out


def _psb(bd_w):
    out = np.zeros((128, 12), np.float32)
    d = bd_w[1, :, 0, 0] - bd_w[0, :, 0, 0]
    for r in range(SB):
        out[97 + r * 5:97 + r * 5 + 5, 6 + r] = d
    return out


def _sel():
    out = np.zeros((128, 96), np.float32)
    for r in range(1, 7):
        for fc in range(16):
            out[r * 16 + fc, fc] = 1.0 / NPIX
    return out


def _w1l(se_w1):
    out = np.zeros((128, 96), np.float32)
    out[:16, :16] = se_w1.T
    return out


def _w2r(se_w2):
    out = np.zeros((128, 128), np.float32)
    for r in range(SB):
        out[:16, 97 + r * 5:97 + r * 5 + 5] = se_w2.T
    return out


def _lcrep():
    out = np.zeros((128, 96), np.float32)
    for i in range(SB):
        out[i, i * 16:i * 16 + 16] = 1.0
        out[6 + i, i * 16:i * 16 + 16] = 1.0
    return out


def _pack_w8(fuse_w, fuse_b, fc_w):
    blocks = [_lf(fuse_w, fuse_b, "first").reshape(128, 256),
              _lf(fuse_w, fuse_b, "mid").reshape(128, 256),
              _lf(fuse_w, fuse_b, "last").reshape(128, 256)]
    wcf = _wc(fc_w, True)
    blocks += [wcf[dx].reshape(128, 192) for dx in range(3)]
    wcx = _wc(fc_w, False)
    blocks += [wcx[dx].reshape(128, 192) for dx in range(3)]
    return np.concatenate(blocks, axis=1).astype(F8)


def _pack_wb(se_w1, se_w2, fm_w, fm_b, bd_w, bd_b):
    blocks = [_sel(), _w1l(se_w1), _w2r(se_w2), _lcrep(),
              _lm(fm_w, fm_b, bd_b), _psb(bd_w)]
    return np.concatenate(blocks, axis=1).astype(BF16)


# ----------------------------------------------------------------------------
# bass graph
# ----------------------------------------------------------------------------

def _build():
    import concourse.bass as bass
    import concourse.bacc as bacc
    import concourse.tile as tile
    from concourse import mybir

    f32 = mybir.dt.float32
    bf16 = mybir.dt.bfloat16
    fp8 = mybir.dt.float8e4
    AF = mybir.ActivationFunctionType
    ALU = mybir.AluOpType
    DR = mybir.MatmulPerfMode.DoubleRow

    def _mid(v, stride, n):
        """Insert a middle dim [stride, n] into a [P, C] view -> [P, n, C]."""
        return bass.AP(v.tensor, v.offset,
                       [list(v.ap[0]), [stride, n], list(v.ap[1])])

    nc = bacc.Bacc("TRN2", target_bir_lowering=False)
    xp_ext = nc.declare_dram_parameter("xp", [NT, 128, W], fp8, isOutput=False)
    yh_ext = nc.declare_dram_parameter("yhp", [NT, 20, 1024], fp8,
                                       isOutput=False)
    yc_ext = nc.declare_dram_parameter("ycp", [NT, 30, W], bf16,
                                       isOutput=False)
    w8_ext = nc.declare_dram_parameter("w8", [128, W8C], fp8, isOutput=False)
    wb_ext = nc.declare_dram_parameter("wb", [128, WBC], bf16, isOutput=False)
    fcb_ext = nc.declare_dram_parameter("fcb", [96, 1], f32, isOutput=False)
    cvw_ext = nc.declare_dram_parameter("cvw", [96, 4], f32, isOutput=False)
    c8_ext = nc.declare_dram_parameter("c8", [108, 1024], fp8, isOutput=False)
    cb_ext = nc.declare_dram_parameter("cb", [2, NT * W], bf16,
                                   isOutput=False)
    out_ext = nc.declare_dram_parameter("out", [NT, 96, W], bf16,
                                        isOutput=True)

    with tile.TileContext(nc) as tc:
        with tc.tile_pool(name="singles", bufs=1) as singles:
            # ---------------- persistent SBUF ----------------
            big = singles.tile([128, (NT + XSLOTS) * PITCH], fp8, tag="big")
            fccbig = singles.tile([128, NT * W], bf16, tag="fccbig")
            yht = singles.tile([128, YSLOTS * 1024], fp8, tag="yht")
            w8 = singles.tile([128, W8C], fp8, tag="w8")
            wb = singles.tile([128, WBC], bf16, tag="wb")
            fcb = singles.tile([96, 1], f32, tag="fcb")
            cvw = singles.tile([96, 4], f32, tag="cvw")
            Ra = singles.tile([128, NT], f32, tag="Ra")
            sigring = singles.tile([128, 4 * 2048], bf16, tag="sigring")
            rept = singles.tile([96, 4 * 2048], bf16, tag="rept")
            LBM = singles.tile([128, 12], bf16, tag="lbm")

            nc.sync.dma_start(out=w8[:, :], in_=w8_ext[:, :])
            nc.sync.dma_start(out=wb[:, :], in_=wb_ext[:, :])
            nc.sync.dma_start(out=fcb[:, :], in_=fcb_ext[:, :])
            nc.sync.dma_start(out=cvw[:, :], in_=cvw_ext[:, :])

            LF_F = _mid(w8[:, 0:128], 128, 2)
            LF = _mid(w8[:, 256:384], 128, 2)
            LF_L = _mid(w8[:, 512:640], 128, 2)
            WCF = [_mid(w8[:, 768 + dx * 192:768 + dx * 192 + 96], 96, 2)
                   for dx in range(3)]
            WCX = [_mid(w8[:, 1344 + dx * 192:1344 + dx * 192 + 96], 96, 2)
                   for dx in range(3)]
            SEL = wb[:, 0:96]
            W1L = wb[:, 96:192]
            W2R = wb[:, 192:320]
            LCREP = wb[:, 320:416]
            LM = wb[:, 416:428]
            PSB = wb[:, 428:440]

            # guard cols of F region and x ring (cols 0 and 513 of each slot)
            gv = big[:, 513:515]
            nc.vector.memset(
                bass.AP(gv.tensor, gv.offset,
                        [list(gv.ap[0]), [PITCH, NT + XSLOTS - 1], [1, 2]]),
                0.0)
            nc.vector.memset(big[:, 0:1], 0.0)
            nc.vector.memset(big[:, (NT + XSLOTS) * PITCH - 1:
                                 (NT + XSLOTS) * PITCH], 0.0)

            # yht slot 0..3 presets up front; the other 12 slots and all
            # phase-2 presets are issued behind the first input batches
            c8v = c8_ext[:, :]
            nc.sync.dma_start(
                out=yht[20:128, 0:4 * 1024],
                in_=bass.AP(c8v.tensor, c8v.offset,
                            [list(c8v.ap[0]), [0, 4], list(c8v.ap[1])]))

            def late_presets(stage):
                if stage < 3:
                    # yht slots 4..7 / 8..11 / 12..15
                    lo = 4 + 4 * stage
                    nc.sync.dma_start(
                        out=yht[20:128, lo * 1024:(lo + 4) * 1024],
                        in_=bass.AP(c8v.tensor, c8v.offset,
                                    [list(c8v.ap[0]), [0, 4],
                                     list(c8v.ap[1])]))
                elif stage == 3:
                    nc.sync.dma_start(out=fccbig[96:97, :], in_=cb_ext[0:1, :])
                    nc.sync.dma_start(out=fccbig[127:128, :],
                                      in_=cb_ext[1:2, :])
                else:
                    z4 = cb_ext[1:2, 0:8192]
                    nc.sync.dma_start(
                        out=sigring[12:128, :],
                        in_=bass.AP(z4.tensor, z4.offset,
                                    [list(z4.ap[0]), [0, 116],
                                     list(z4.ap[1])]))

            # ========== phase 1: fuse + conv fronts, then SE ============
            ctx2 = tc.tile_pool(name="psC", bufs=2, space="PSUM")
            psC = ctx2.__enter__()
            with tc.tile_pool(name="psF", bufs=2, space="PSUM") as psF:
                fhold = {}

                def fuse(t):
                    if t % 4 == 0:
                        n = min(4, NT - t)
                        k0 = t % YSLOTS
                        nc.sync.dma_start(
                            out=yht[0:20, k0 * 1024:(k0 + n) * 1024],
                            in_=yh_ext[t:t + n, :, :].rearrange(
                                "s p j -> p s j"))
                        if t <= 12:
                            late_presets(t // 4)
                        elif t == 56:
                            late_presets(4)
                    yslot = yht[:, (t % YSLOTS) * 1024:
                                (t % YSLOTS) * 1024 + 512]
                    lf = LF_F if t == 0 else (LF_L if t == NT - 1 else LF)
                    if t % 2 == 0:
                        fhold["t"] = psF.tile([128, 1024], f32, tag="fuse",
                                              name="fusepair")
                    fps = fhold["t"]
                    h = (t % 2) * W
                    nc.tensor.matmul(fps[:, h:h + W], lhsT=lf,
                                     rhs=_mid(yslot, 512, 2),
                                     start=True, stop=True, perf_mode=DR)
                    if t % 2 == 1:
                        c0 = _fcol(t - 1) + 1
                        d = _fcol(t) - _fcol(t - 1)
                        fv = big[:, c0:c0 + W]
                        fdst = bass.AP(fv.tensor, fv.offset,
                                       [list(fv.ap[0]), [d, 2], [1, W]])
                        nc.vector.tensor_scalar(
                            out=fdst, in0=fps[:, :], scalar1=0.0, scalar2=0.0,
                            op0=ALU.max, op1=ALU.add,
                            accum_out=Ra[:, t // 2:t // 2 + 1])

                def front(s):
                    if s % 4 == 0:
                        n = min(4, NT - s)
                        j0 = s % XSLOTS
                        xc0 = (XMID + j0) * PITCH
                        dst = big[:, xc0 + 1:xc0 + 1 + W]
                        nc.sync.dma_start(
                            out=bass.AP(dst.tensor, dst.offset,
                                        [list(dst.ap[0]), [PITCH, n], [1, W]]),
                            in_=xp_ext[s:s + n, :, :].rearrange(
                                "s p j -> p s j"))
                        if s % 8 == 0:
                            n2 = min(8, NT - s)
                            nc.sync.dma_start(
                                out=fccbig[97:127, s * W:(s + n2) * W],
                                in_=yc_ext[s:s + n2, :, :].rearrange(
                                    "s p j -> p s j"))
                    fcol = _fcol(s)
                    xcol = (XMID + s % XSLOTS) * PITCH
                    base, delta = ((fcol, xcol - fcol) if s < XMID
                                   else (xcol, fcol - xcol))
                    wcs = WCF if s < XMID else WCX
                    if s % 2 == 0:
                        fhold["s"] = psC.tile([96, 1024], f32, tag="conv",
                                              name="convpair")
                    cps = fhold["s"]
                    hh = (s % 2) * W
                    for dx in range(3):
                        v = big[:, base + dx:base + dx + W]
                        nc.tensor.matmul(cps[:, hh:hh + W], lhsT=wcs[dx],
                                         rhs=_mid(v, delta, 2),
                                         start=(dx == 0), stop=(dx == 2),
                                         perf_mode=DR)
                    if s % 2 == 1:
                        fdst = fccbig[0:96, (s - 1) * W:(s + 1) * W]
                        nc.scalar.activation(
                            out=fdst, in_=cps[:, :], func=AF.Relu,
                            bias=fcb[:, :])

                for i in range(NT + FLAG):
                    if i < NT:
                        fuse(i)
                    if i >= FLAG:
                        front(i - FLAG)

                # ================= SE chain ==============================
                NP2 = NT // 2
                Rbf = singles.tile([128, NP2], bf16, tag="Rbf")
                nc.vector.tensor_copy(out=Rbf[:, :], in_=Ra[:, 0:NP2])
                gps = psF.tile([96, NP2], f32, tag="fuse")
                nc.tensor.matmul(gps[:, :], lhsT=SEL, rhs=Rbf[:, :],
                                 start=True, stop=True)
                gap_f = singles.tile([96, 1], f32, tag="gapf")
                nc.vector.reduce_sum(out=gap_f[:, :], in_=gps[:, :],
                                     axis=mybir.AxisListType.X)
                gap_bf = singles.tile([128, 1], bf16, tag="gap")
                nc.vector.memset(gap_bf[:, :], 0.0)
                nc.vector.tensor_copy(out=gap_bf[0:96, :], in_=gap_f[:, :])
                hps = psF.tile([96, 1], f32, tag="fuse")
                nc.tensor.matmul(hps[:, :], lhsT=W1L, rhs=gap_bf[:, :],
                                 start=True, stop=True)
                h_bf = singles.tile([128, 1], bf16, tag="hbf")
                nc.vector.memset(h_bf[:, :], 0.0)
                nc.scalar.activation(out=h_bf[0:96, :], in_=hps[:, :],
                                     func=AF.Relu)
                sps = psF.tile([128, 1], f32, tag="fuse")
                nc.tensor.matmul(sps[:, :], lhsT=W2R, rhs=h_bf[:, :],
                                 start=True, stop=True)
                se_bc = singles.tile([128, 1], f32, tag="sebc")
                nc.scalar.activation(out=se_bc[:, :], in_=sps[:, :],
                                     func=AF.Sigmoid)
                nc.vector.scalar_tensor_tensor(
                    out=LBM[:, :], in0=PSB, scalar=se_bc[:, :], in1=LM,
                    op0=ALU.mult, op1=ALU.add)

            ctx2.__exit__(None, None, None)
            # ========== phase 2: tails ==================================
            with (
                tc.tile_pool(name="psM", bufs=2, space="PSUM") as psM,
                tc.tile_pool(name="psV", bufs=2, space="PSUM") as psV,
            ):
                def tail_block(b):
                    u0 = b * 4
                    ns = min(4, NT - u0)
                    w4 = ns * W
                    sslot = (b % 4) * 2048
                    npair = (ns + 1) // 2
                    for p in range(npair):
                        u = u0 + 2 * p
                        np_ = min(2, NT - u)
                        mps = psM.tile([12, 1024], f32, tag="mb")
                        for q in range(np_):
                            nc.tensor.matmul(
                                mps[:, q * W:(q + 1) * W], lhsT=LBM[:, :],
                                rhs=fccbig[:, (u + q) * W:(u + q + 1) * W],
                                start=True, stop=True)
                        nc.scalar.activation(
                            out=sigring[0:12, sslot + p * 1024:
                                        sslot + p * 1024 + np_ * W],
                            in_=mps[:, 0:np_ * W], func=AF.Sigmoid)
                    # v = sigma_m + sigma_b, replicated x16 via one matmul;
                    # evacuation folds min(v,1)*cv_w; then +cv_b in 4x mode.
                    rp = rept[:, (b % 4) * 2048:(b % 4) * 2048 + w4]
                    aform = (b % 4 == 0)
                    for q in range(ns):
                        vps = psV.tile([96, W], f32, tag="cv")
                        nc.tensor.matmul(
                            vps[:, :], lhsT=LCREP,
                            rhs=sigring[:, sslot + q * W:sslot + (q + 1) * W],
                            start=True, stop=True)
                        og = rept[:, (b % 4) * 2048 + q * W:
                                  (b % 4) * 2048 + (q + 1) * W]
                        if aform:
                            # r = relu(1 - v) on Act; min(v,1) = 1 - r
                            nc.scalar.activation(
                                out=og, in_=vps[:, :], func=AF.Relu,
                                scale=-1.0, bias=1.0)
                        else:
                            nc.vector.tensor_scalar(
                                out=og, in0=vps[:, :], scalar1=1.0,
                                scalar2=cvw[:, 0:1], op0=ALU.min,
                                op1=ALU.mult)
                    if aform:
                        # out = -w*r + (w+b)
                        nc.vector.tensor_scalar(
                            out=rp, in0=rp, scalar1=cvw[:, 2:3],
                            scalar2=cvw[:, 3:4], op0=ALU.mult, op1=ALU.add)
                    elif b >= NBLK - 3:
                        nc.vector.tensor_scalar(
                            out=rp, in0=rp, scalar1=cvw[:, 1:2],
                            scalar2=None, op0=ALU.add)
                    else:
                        nc.gpsimd.tensor_scalar(
                            out=rp, in0=rp, scalar1=cvw[:, 1:2],
                            scalar2=None, op0=ALU.add)
                    nc.sync.dma_start(
                        out=out_ext[u0:u0 + ns, :, :].rearrange(
                            "s p j -> p s j"),
                        in_=rp)

                for b in range(NBLK):
                    tail_block(b)
    nc.compile()
    return nc


# ----------------------------------------------------------------------------
# entry point
# ----------------------------------------------------------------------------

LAST_RESULT = None


def prepare(x, y, fuse_w, fuse_b, se_w1, se_w2, bd_w, bd_b,
            fc_w, fc_b, fm_w, fm_b, cv_w, cv_b):
    if "nc" not in _cache:
        _cache["nc"] = _build()
    nc = _cache["nc"]

    g = {k: np.asarray(v, np.float32) for k, v in dict(
        fuse_w=fuse_w, fuse_b=fuse_b, se_w1=se_w1, se_w2=se_w2, bd_w=bd_w,
        bd_b=bd_b, fc_w=fc_w, fc_b=fc_b, fm_w=fm_w, fm_b=fm_b, cv_w=cv_w,
        cv_b=cv_b).items()}
    w8 = _pack_w8(g["fuse_w"], g["fuse_b"], g["fc_w"])
    wb = _pack_wb(g["se_w1"], g["se_w2"], g["fm_w"], g["fm_b"],
                  g["bd_w"], g["bd_b"])
    fcb = np.zeros((96, 1), np.float32)
    cvw = np.zeros((96, 4), np.float32)
    for i in range(SB):
        fcb[i * 16:(i + 1) * 16, 0] = g["fc_b"]
        cvw[i * 16:(i + 1) * 16, 0] = g["cv_w"][:, 0, 0, 0]
        cvw[i * 16:(i + 1) * 16, 1] = g["cv_b"]
    cvw[:, 2] = -cvw[:, 0]
    cvw[:, 3] = cvw[:, 0] + cvw[:, 1]

    c8 = np.zeros((108, 1024), F8)       # partitions 20..127 of a yht slot
    c8[0, 0:512] = 1.0                   # ones partition (20), ktile0 only
    cb = np.zeros((2, NT * W), BF16)
    cb[0, :] = 1.0                       # ones row; row 1 zeros

    xb = np.asarray(x, np.float32)
    yb = np.asarray(y, np.float32)
    B = xb.shape[0]

    xpad = np.zeros((B, 16, 6 * NT + 8, W), np.float32)
    xpad[:, :, 1:H + 1, :] = xb
    ridx = 6 * np.arange(NT)[:, None] + np.arange(8)[None, :]
    xp = xpad[:, :, ridx, :].transpose(0, 2, 3, 1, 4) \
        .reshape(B, NT, 128, W).astype(F8)

    ypad = np.zeros((B, 5, 6 * NT + 8, W), np.float32)
    ypad[:, :, 1:H + 1, :] = yb
    # yhp[b,t, r'*5+c, k*512+n] = ypad[b, c, 6t+4k+r', n]
    yidx = (6 * np.arange(NT)[:, None, None]
            + 4 * np.arange(2)[None, :, None]
            + np.arange(4)[None, None, :])
    yhp = ypad[:, :, yidx, :]                     # [B, 5, NT, 2, 4, W]
    yhp = yhp.transpose(0, 2, 4, 1, 3, 5).reshape(B, NT, 20, 2 * W).astype(F8)

    cidx = 6 * np.arange(NT)[:, None] + 1 + np.arange(6)[None, :]
    ycp = ypad[:, :, cidx, :].transpose(0, 2, 3, 1, 4) \
        .reshape(B, NT, 30, W).astype(BF16)

    in_maps = [
        {"xp": np.ascontiguousarray(xp[i]),
         "yhp": np.ascontiguousarray(yhp[i]),
         "ycp": np.ascontiguousarray(ycp[i]),
         "w8": w8, "wb": wb, "fcb": fcb, "cvw": cvw, "c8": c8, "cb": cb}
        for i in range(B)
    ]
    return nc, in_maps


def kernel(x, y, fuse_w, fuse_b, se_w1, se_w2, bd_w, bd_b,
           fc_w, fc_b, fm_w, fm_b, cv_w, cv_b):
    global LAST_RESULT
    from concourse.bass_utils import run_bass_kernel_spmd

    nc, in_maps = prepare(x, y, fuse_w, fuse_b, se_w1, se_w2, bd_w, bd_b,
                          fc_w, fc_b, fm_w, fm_b, cv_w, cv_b)
    res = run_bass_kernel_spmd(nc, in_maps, core_ids=list(range(8)))
    LAST_RESULT = res
    outs = []
    for i in range(len(in_maps)):
        ot = np.asarray(res.results[i]["out"], np.float32)
        full = ot.reshape(NT, SB, 16, W).transpose(2, 0, 1, 3) \
                 .reshape(16, NT * SB, W)[:, :H, :]
        outs.append(full)
    return np.stack(outs)


# revision 6
# speedup vs baseline: 1.0709x; 1.0142x over previous
"""Trainium2 Bass kernel v2 for nn_Boundary_Enchance (dense_cnn).

Data parallel: core i processes batch image i.  The heavy matmuls (fuse 1x1
conv and the 3x3 conv) run as fp8e4 DoubleRow matmuls: two K=128 k-tiles per
instruction at 0.5 cycles per output column -- 4x the bf16 rate.  End-to-end
fp8 error measured at ~3e-3 (budget 2e-2).

Per-core structure:
  prebarrier - per 8-row tile t (stride 6): fuse_box = relu(1x1conv(y)+b) as
    one DoubleRow matmul (y rows 0..3 = k-tile 0, rows 4..7 = k-tile 1, bias
    via a preset ones partition).  Evacuation (rotating DVE/Act/Pool) writes
    the persistent fp8 F region and per-tile row sums (accum_out).
  SE chain - row sums -> selection matmul -> gap -> MLP -> sigmoid ->
    LBM = PSB*se + LM (data-dependent mask+boundary lhsT, M=12).
  main loop - fronts run 4+ strips ahead of 4-strip tail blocks:
    front: conv3x3 over concat(F, x) as 3 DoubleRow matmuls (k-tile 0 = F
      region, k-tile 1 = x ring; dx taps via guard-column shifted views, all
      full-width);  fcc = relu(conv+b) evacuated bf16 into the persistent
      fccbig region (y rides at partitions 97..126 for the boundary head).
    tail block (4 strips): mask+boundary logits as bf16 matmuls into [12,
      1024] PSUM pairs, one sigmoid per pair, batched add+min -> scale s;
      out = cv_w*s + cv_b via a replication matmul (M=96) + one affine
      tensor_scalar during the PSUM->SBUF evacuation; bf16 output DMA.
"""

import numpy as np
import ml_dtypes

BF16 = ml_dtypes.bfloat16
F8 = ml_dtypes.float8_e4m3

H = 512
W = 512
SB = 6
NT = (H + SB - 1) // SB          # 86 strips
NPIX = float(H * W)
PITCH = W + 2                    # F / x slot pitch (guard cols at 0, 513)
XSLOTS = 16
YSLOTS = 16
XMID = 43                        # x-ring sits between F(42) and F(43)
W8C = 3 * 256 + 6 * 192          # w8 cols: LF_F|LF|LF_L + WCF0..2 + WCX0..2
WBC = 96 + 96 + 128 + 96 + 12 + 12  # SEL W1L W2R LCREP LM PSB
NBLK = (NT + 3) // 4             # 4-strip tail blocks (last partial)
FLAG = 4

_cache = {}


# ----------------------------------------------------------------------------
# host-side weight layout builders
# ----------------------------------------------------------------------------

def _fcol(s):
    return (s if s < XMID else s + XSLOTS) * PITCH


def _lf(fuse_w, fuse_b, variant):
    """[128, 2, 128] fuse lhsT.  partition p=r'*5+c (r'<4), ones at p=20.
    k-tile k covers tile rows r=4k+r'.  out col m=r*16+oc."""
    out = np.zeros((128, 2, 128), np.float32)
    fw = fuse_w[:, :, 0, 0]                       # [16 oc, 5 ic]
    for k in range(2):
        for rp in range(4):
            r = 4 * k + rp
            for c in range(5):
                out[rp * 5 + c, k, r * 16:r * 16 + 16] = fw[:, c]
    out[20, 0, :] = np.tile(fuse_b, 8)            # bias on ones partition
    if variant == "first":
        out[:, :, 0:16] = 0.0
    elif variant == "last":
        out[:, :, 48:128] = 0.0
    return out


def _wc(fc_w, forder):
    """[3][128, 2, 96] conv lhsT per dx.  p=r*16+c; k-tiles = (F half, x
    half) if forder else (x half, F half); out col m=i*16+oc, taps 0..2."""
    out = np.zeros((3, 128, 2, 96), np.float32)
    kf, kx = (0, 1) if forder else (1, 0)
    for dx in range(3):
        for i in range(SB):
            for ky in range(3):
                r = i + ky
                out[dx, r * 16:r * 16 + 16, kf, i * 16:i * 16 + 16] = \
                    fc_w[:, 16:32, ky, dx].T
                out[dx, r * 16:r * 16 + 16, kx, i * 16:i * 16 + 16] = \
                    fc_w[:, 0:16, ky, dx].T
    return out


def _lm(fm_w, fm_b, bd_b):
    out = np.zeros((128, 12), np.float32)
    d = fm_w[1, :, 0, 0] - fm_w[0, :, 0, 0]
    for i in range(SB):
        out[i * 16:i * 16 + 16, i] = d
    out[96, 0:SB] = fm_b[1] - fm_b[0]
    out[96, 6:6 + SB] = bd_b[1] - bd_b[0]
    return out


def _psb(bd_w):
    out = np.zeros((128, 12), np.float32)
    d = bd_w[1, :, 0, 0] - bd_w[0, :, 0, 0]
    for r in range(SB):
        out[97 + r * 5:97 + r * 5 + 5, 6 + r] = d
    return out


def _sel():
    out = np.zeros((128, 96), np.float32)
    for r in range(1, 7):
        for fc in range(16):
            out[r * 16 + fc, fc] = 1.0 / NPIX
    return out


def _w1l(se_w1):
    out = np.zeros((128, 96), np.float32)
    out[:16, :16] = se_w1.T
    return out


def _w2r(se_w2):
    out = np.zeros((128, 128), np.float32)
    for r in range(SB):
        out[:16, 97 + r * 5:97 + r * 5 + 5] = se_w2.T
    return out


def _lcrep():
    out = np.zeros((128, 96), np.float32)
    for i in range(SB):
        out[i, i * 16:i * 16 + 16] = 1.0
        out[6 + i, i * 16:i * 16 + 16] = 1.0
    return out


def _pack_w8(fuse_w, fuse_b, fc_w):
    blocks = [_lf(fuse_w, fuse_b, "first").reshape(128, 256),
              _lf(fuse_w, fuse_b, "mid").reshape(128, 256),
              _lf(fuse_w, fuse_b, "last").reshape(128, 256)]
    wcf = _wc(fc_w, True)
    blocks += [wcf[dx].reshape(128, 192) for dx in range(3)]
    wcx = _wc(fc_w, False)
    blocks += [wcx[dx].reshape(128, 192) for dx in range(3)]
    return np.concatenate(blocks, axis=1).astype(F8)


def _pack_wb(se_w1, se_w2, fm_w, fm_b, bd_w, bd_b):
    blocks = [_sel(), _w1l(se_w1), _w2r(se_w2), _lcrep(),
              _lm(fm_w, fm_b, bd_b), _psb(bd_w)]
    return np.concatenate(blocks, axis=1).astype(BF16)


# ----------------------------------------------------------------------------
# bass graph
# ----------------------------------------------------------------------------

def _build():
    import concourse.bass as bass
    import concourse.bacc as bacc
    import concourse.tile as tile
    from concourse import mybir

    f32 = mybir.dt.float32
    bf16 = mybir.dt.bfloat16
    fp8 = mybir.dt.float8e4
    AF = mybir.ActivationFunctionType
    ALU = mybir.AluOpType
    DR = mybir.MatmulPerfMode.DoubleRow

    def _mid(v, stride, n):
        """Insert a middle dim [stride, n] into a [P, C] view -> [P, n, C]."""
        return bass.AP(v.tensor, v.offset,
                       [list(v.ap[0]), [stride, n], list(v.ap[1])])

    nc = bacc.Bacc("TRN2", target_bir_lowering=False)
    xp_ext = nc.declare_dram_parameter("xp", [NT, 128, W], fp8, isOutput=False)
    yh_ext = nc.declare_dram_parameter("yhp", [NT, 20, 1024], fp8,
                                       isOutput=False)
    yc_ext = nc.declare_dram_parameter("ycp", [NT, 30, W], bf16,
                                       isOutput=False)
    w8_ext = nc.declare_dram_parameter("w8", [128, W8C], fp8, isOutput=False)
    wb_ext = nc.declare_dram_parameter("wb", [128, WBC], bf16, isOutput=False)
    fcb_ext = nc.declare_dram_parameter("fcb", [96, 1], f32, isOutput=False)
    cvw_ext = nc.declare_dram_parameter("cvw", [96, 4], f32, isOutput=False)
    c8_ext = nc.declare_dram_parameter("c8", [108, 1024], fp8, isOutput=False)
    cb_ext = nc.declare_dram_parameter("cb", [2, NT * W], bf16,
                                   isOutput=False)
    out_ext = nc.declare_dram_parameter("out", [NT, 96, W], bf16,
                                        isOutput=True)

    with tile.TileContext(nc) as tc:
        with tc.tile_pool(name="singles", bufs=1) as singles:
            # ---------------- persistent SBUF ----------------
            big = singles.tile([128, (NT + XSLOTS) * PITCH], fp8, tag="big")
            fccbig = singles.tile([128, NT * W], bf16, tag="fccbig")
            yht = singles.tile([128, YSLOTS * 1024], fp8, tag="yht")
            w8 = singles.tile([128, W8C], fp8, tag="w8")
            wb = singles.tile([128, WBC], bf16, tag="wb")
            fcb = singles.tile([96, 1], f32, tag="fcb")
            cvw = singles.tile([96, 4], f32, tag="cvw")
            Ra = singles.tile([128, NT], f32, tag="Ra")
            sigring = singles.tile([128, 4 * 2048], bf16, tag="sigring")
            rept = singles.tile([96, 4 * 2048], bf16, tag="rept")
            LBM = singles.tile([128, 12], bf16, tag="lbm")

            nc.sync.dma_start(out=w8[:, :], in_=w8_ext[:, :])
            nc.sync.dma_start(out=wb[:, :], in_=wb_ext[:, :])
            nc.sync.dma_start(out=fcb[:, :], in_=fcb_ext[:, :])
            nc.sync.dma_start(out=cvw[:, :], in_=cvw_ext[:, :])

            LF_F = _mid(w8[:, 0:128], 128, 2)
            LF = _mid(w8[:, 256:384], 128, 2)
            LF_L = _mid(w8[:, 512:640], 128, 2)
            WCF = [_mid(w8[:, 768 + dx * 192:768 + dx * 192 + 96], 96, 2)
                   for dx in range(3)]
            WCX = [_mid(w8[:, 1344 + dx * 192:1344 + dx * 192 + 96], 96, 2)
                   for dx in range(3)]
            SEL = wb[:, 0:96]
            W1L = wb[:, 96:192]
            W2R = wb[:, 192:320]
            LCREP = wb[:, 320:416]
            LM = wb[:, 416:428]
            PSB = wb[:, 428:440]

            # guard cols of F region and x ring (cols 0 and 513 of each slot)
            gv = big[:, 513:515]
            nc.vector.memset(
                bass.AP(gv.tensor, gv.offset,
                        [list(gv.ap[0]), [PITCH, NT + XSLOTS - 1], [1, 2]]),
                0.0)
            nc.vector.memset(big[:, 0:1], 0.0)
            nc.vector.memset(big[:, (NT + XSLOTS) * PITCH - 1:
                                 (NT + XSLOTS) * PITCH], 0.0)

            # yht slot 0..3 presets up front; the other 12 slots and all
            # phase-2 presets are issued behind the first input batches
            c8v = c8_ext[:, :]
            nc.sync.dma_start(
                out=yht[20:128, 0:4 * 1024],
                in_=bass.AP(c8v.tensor, c8v.offset,
                            [list(c8v.ap[0]), [0, 4], list(c8v.ap[1])]))

            def late_presets(stage):
                if stage < 3:
                    # yht slots 4..7 / 8..11 / 12..15
                    lo = 4 + 4 * stage
                    nc.sync.dma_start(
                        out=yht[20:128, lo * 1024:(lo + 4) * 1024],
                        in_=bass.AP(c8v.tensor, c8v.offset,
                                    [list(c8v.ap[0]), [0, 4],
                                     list(c8v.ap[1])]))
                elif stage == 3:
                    nc.sync.dma_start(out=fccbig[96:97, :], in_=cb_ext[0:1, :])
                    nc.sync.dma_start(out=fccbig[127:128, :],
                                      in_=cb_ext[1:2, :])
                else:
                    z4 = cb_ext[1:2, 0:8192]
                    nc.sync.dma_start(
                        out=sigring[12:128, :],
                        in_=bass.AP(z4.tensor, z4.offset,
                                    [list(z4.ap[0]), [0, 116],
                                     list(z4.ap[1])]))

            # ========== phase 1: fuse + conv fronts, then SE ============
            ctx2 = tc.tile_pool(name="psC", bufs=2, space="PSUM")
            psC = ctx2.__enter__()
            with tc.tile_pool(name="psF", bufs=2, space="PSUM") as psF:
                fhold = {}

                def fuse(t):
                    if t % 4 == 0:
                        n = min(4, NT - t)
                        k0 = t % YSLOTS
                        nc.sync.dma_start(
                            out=yht[0:20, k0 * 1024:(k0 + n) * 1024],
                            in_=yh_ext[t:t + n, :, :].rearrange(
                                "s p j -> p s j"))
                        if t <= 12:
                            late_presets(t // 4)
                        elif t == 56:
                            late_presets(4)
                    yslot = yht[:, (t % YSLOTS) * 1024:
                                (t % YSLOTS) * 1024 + 512]
                    lf = LF_F if t == 0 else (LF_L if t == NT - 1 else LF)
                    if t % 2 == 0:
                        fhold["t"] = psF.tile([128, 1024], f32, tag="fuse",
                                              name="fusepair")
                    fps = fhold["t"]
                    h = (t % 2) * W
                    nc.tensor.matmul(fps[:, h:h + W], lhsT=lf,
                                     rhs=_mid(yslot, 512, 2),
                                     start=True, stop=True, perf_mode=DR)
                    if t % 2 == 1:
                        c0 = _fcol(t - 1) + 1
                        d = _fcol(t) - _fcol(t - 1)
                        fv = big[:, c0:c0 + W]
                        fdst = bass.AP(fv.tensor, fv.offset,
                                       [list(fv.ap[0]), [d, 2], [1, W]])
                        nc.vector.tensor_scalar(
                            out=fdst, in0=fps[:, :], scalar1=0.0, scalar2=0.0,
                            op0=ALU.max, op1=ALU.add,
                            accum_out=Ra[:, t // 2:t // 2 + 1])

                def front(s):
                    if s % 4 == 0:
                        n = min(4, NT - s)
                        j0 = s % XSLOTS
                        xc0 = (XMID + j0) * PITCH
                        dst = big[:, xc0 + 1:xc0 + 1 + W]
                        nc.sync.dma_start(
                            out=bass.AP(dst.tensor, dst.offset,
                                        [list(dst.ap[0]), [PITCH, n], [1, W]]),
                            in_=xp_ext[s:s + n, :, :].rearrange(
                                "s p j -> p s j"))
                        if s % 8 == 0:
                            n2 = min(8, NT - s)
                            nc.sync.dma_start(
                                out=fccbig[97:127, s * W:(s + n2) * W],
                                in_=yc_ext[s:s + n2, :, :].rearrange(
                                    "s p j -> p s j"))
                    fcol = _fcol(s)
                    xcol = (XMID + s % XSLOTS) * PITCH
                    base, delta = ((fcol, xcol - fcol) if s < XMID
                                   else (xcol, fcol - xcol))
                    wcs = WCF if s < XMID else WCX
                    if s % 2 == 0:
                        fhold["s"] = psC.tile([96, 1024], f32, tag="conv",
                                              name="convpair")
                    cps = fhold["s"]
                    hh = (s % 2) * W
                    for dx in range(3):
                        v = big[:, base + dx:base + dx + W]
                        nc.tensor.matmul(cps[:, hh:hh + W], lhsT=wcs[dx],
                                         rhs=_mid(v, delta, 2),
                                         start=(dx == 0), stop=(dx == 2),
                                         perf_mode=DR)
                    if s % 2 == 1:
                        fdst = fccbig[0:96, (s - 1) * W:(s + 1) * W]
                        nc.scalar.activation(
                            out=fdst, in_=cps[:, :], func=AF.Relu,
                            bias=fcb[:, :])

                for i in range(NT + FLAG):
                    if i < NT:
                        fuse(i)
                    if i >= FLAG:
                        front(i - FLAG)

                # ================= SE chain ==============================
                NP2 = NT // 2
                Rbf = singles.tile([128, NP2], bf16, tag="Rbf")
                nc.vector.tensor_copy(out=Rbf[:, :], in_=Ra[:, 0:NP2])
                gps = psF.tile([96, NP2], f32, tag="fuse")
                nc.tensor.matmul(gps[:, :], lhsT=SEL, rhs=Rbf[:, :],
                                 start=True, stop=True)
                gap_f = singles.tile([96, 1], f32, tag="gapf")
                nc.vector.reduce_sum(out=gap_f[:, :], in_=gps[:, :],
                                     axis=mybir.AxisListType.X)
                gap_bf = singles.tile([128, 1], bf16, tag="gap")
                nc.vector.memset(gap_bf[:, :], 0.0)
                nc.vector.tensor_copy(out=gap_bf[0:96, :], in_=gap_f[:, :])
                hps = psF.tile([96, 1], f32, tag="fuse")
                nc.tensor.matmul(hps[:, :], lhsT=W1L, rhs=gap_bf[:, :],
                                 start=True, stop=True)
                h_bf = singles.tile([128, 1], bf16, tag="hbf")
                nc.vector.memset(h_bf[:, :], 0.0)
                nc.scalar.activation(out=h_bf[0:96, :], in_=hps[:, :],
                                     func=AF.Relu)
                sps = psF.tile([128, 1], f32, tag="fuse")
                nc.tensor.matmul(sps[:, :], lhsT=W2R, rhs=h_bf[:, :],
                                 start=True, stop=True)
                se_bc = singles.tile([128, 1], f32, tag="sebc")
                nc.scalar.activation(out=se_bc[:, :], in_=sps[:, :],
                                     func=AF.Sigmoid)
                nc.vector.scalar_tensor_tensor(
                    out=LBM[:, :], in0=PSB, scalar=se_bc[:, :], in1=LM,
                    op0=ALU.mult, op1=ALU.add)

            ctx2.__exit__(None, None, None)
            # ========== phase 2: tails ==================================
            with (
                tc.tile_pool(name="psM", bufs=2, space="PSUM") as psM,
                tc.tile_pool(name="psV", bufs=2, space="PSUM") as psV,
            ):
                def tail_block(b):
                    u0 = b * 4
                    ns = min(4, NT - u0)
                    w4 = ns * W
                    sslot = (b % 4) * 2048
                    npair = (ns + 1) // 2
                    for p in range(npair):
                        u = u0 + 2 * p
                        np_ = min(2, NT - u)
                        mps = psM.tile([12, 1024], f32, tag="mb")
                        for q in range(np_):
                            nc.tensor.matmul(
                                mps[:, q * W:(q + 1) * W], lhsT=LBM[:, :],
                                rhs=fccbig[:, (u + q) * W:(u + q + 1) * W],
                                start=True, stop=True)
                        nc.scalar.activation(
                            out=sigring[0:12, sslot + p * 1024:
                                        sslot + p * 1024 + np_ * W],
                            in_=mps[:, 0:np_ * W], func=AF.Sigmoid)
                    # v = sigma_m + sigma_b, replicated x16 via one matmul;
                    # evacuation folds min(v,1)*cv_w; then +cv_b in 4x mode.
                    rp = rept[:, (b % 4) * 2048:(b % 4) * 2048 + w4]
                    aform = (b % 4 == 0 and b >= 8)
                    for q in range(ns):
                        vps = psV.tile([96, W], f32, tag="cv")
                        nc.tensor.matmul(
                            vps[:, :], lhsT=LCREP,
                            rhs=sigring[:, sslot + q * W:sslot + (q + 1) * W],
                            start=True, stop=True)
                        og = rept[:, (b % 4) * 2048 + q * W:
                                  (b % 4) * 2048 + (q + 1) * W]
                        if aform:
                            # r = relu(1 - v) on Act; min(v,1) = 1 - r
                            nc.scalar.activation(
                                out=og, in_=vps[:, :], func=AF.Relu,
                                scale=-1.0, bias=1.0)
                        else:
                            nc.vector.tensor_scalar(
                                out=og, in0=vps[:, :], scalar1=1.0,
                                scalar2=cvw[:, 0:1], op0=ALU.min,
                                op1=ALU.mult)
                    if aform:
                        # out = -w*r + (w+b)
                        nc.vector.tensor_scalar(
                            out=rp, in0=rp, scalar1=cvw[:, 2:3],
                            scalar2=cvw[:, 3:4], op0=ALU.mult, op1=ALU.add)
                    elif b >= NBLK - 3:
                        nc.vector.tensor_scalar(
                            out=rp, in0=rp, scalar1=cvw[:, 1:2],
                            scalar2=None, op0=ALU.add)
                    else:
                        nc.gpsimd.tensor_scalar(
                            out=rp, in0=rp, scalar1=cvw[:, 1:2],
                            scalar2=None, op0=ALU.add)
                    nc.sync.dma_start(
                        out=out_ext[u0:u0 + ns, :, :].rearrange(
                            "s p j -> p s j"),
                        in_=rp)

                for b in range(NBLK):
                    tail_block(b)
    nc.compile()
    return nc


# ----------------------------------------------------------------------------
# entry point
# ----------------------------------------------------------------------------

LAST_RESULT = None


def prepare(x, y, fuse_w, fuse_b, se_w1, se_w2, bd_w, bd_b,
            fc_w, fc_b, fm_w, fm_b, cv_w, cv_b):
    if "nc" not in _cache:
        _cache["nc"] = _build()
    nc = _cache["nc"]

    g = {k: np.asarray(v, np.float32) for k, v in dict(
        fuse_w=fuse_w, fuse_b=fuse_b, se_w1=se_w1, se_w2=se_w2, bd_w=bd_w,
        bd_b=bd_b, fc_w=fc_w, fc_b=fc_b, fm_w=fm_w, fm_b=fm_b, cv_w=cv_w,
        cv_b=cv_b).items()}
    w8 = _pack_w8(g["fuse_w"], g["fuse_b"], g["fc_w"])
    wb = _pack_wb(g["se_w1"], g["se_w2"], g["fm_w"], g["fm_b"],
                  g["bd_w"], g["bd_b"])
    fcb = np.zeros((96, 1), np.float32)
    cvw = np.zeros((96, 4), np.float32)
    for i in range(SB):
        fcb[i * 16:(i + 1) * 16, 0] = g["fc_b"]
        cvw[i * 16:(i + 1) * 16, 0] = g["cv_w"][:, 0, 0, 0]
        cvw[i * 16:(i + 1) * 16, 1] = g["cv_b"]
    cvw[:, 2] = -cvw[:, 0]
    cvw[:, 3] = cvw[:, 0] + cvw[:, 1]

    c8 = np.zeros((108, 1024), F8)       # partitions 20..127 of a yht slot
    c8[0, 0:512] = 1.0                   # ones partition (20), ktile0 only
    cb = np.zeros((2, NT * W), BF16)
    cb[0, :] = 1.0                       # ones row; row 1 zeros

    xb = np.asarray(x, np.float32)
    yb = np.asarray(y, np.float32)
    B = xb.shape[0]

    xpad = np.zeros((B, 16, 6 * NT + 8, W), np.float32)
    xpad[:, :, 1:H + 1, :] = xb
    ridx = 6 * np.arange(NT)[:, None] + np.arange(8)[None, :]
    xp = xpad[:, :, ridx, :].transpose(0, 2, 3, 1, 4) \
        .reshape(B, NT, 128, W).astype(F8)

    ypad = np.zeros((B, 5, 6 * NT + 8, W), np.float32)
    ypad[:, :, 1:H + 1, :] = yb
    # yhp[b,t, r'*5+c, k*512+n] = ypad[b, c, 6t+4k+r', n]
    yidx = (6 * np.arange(NT)[:, None, None]
            + 4 * np.arange(2)[None, :, None]
            + np.arange(4)[None, None, :])
    yhp = ypad[:, :, yidx, :]                     # [B, 5, NT, 2, 4, W]
    yhp = yhp.transpose(0, 2, 4, 1, 3, 5).reshape(B, NT, 20, 2 * W).astype(F8)

    cidx = 6 * np.arange(NT)[:, None] + 1 + np.arange(6)[None, :]
    ycp = ypad[:, :, cidx, :].transpose(0, 2, 3, 1, 4) \
        .reshape(B, NT, 30, W).astype(BF16)

    in_maps = [
        {"xp": np.ascontiguousarray(xp[i]),
         "yhp": np.ascontiguousarray(yhp[i]),
         "ycp": np.ascontiguousarray(ycp[i]),
         "w8": w8, "wb": wb, "fcb": fcb, "cvw": cvw, "c8": c8, "cb": cb}
        for i in range(B)
    ]
    return nc, in_maps


def kernel(x, y, fuse_w, fuse_b, se_w1, se_w2, bd_w, bd_b,
           fc_w, fc_b, fm_w, fm_b, cv_w, cv_b):
    global LAST_RESULT
    from concourse.bass_utils import run_bass_kernel_spmd

    nc, in_maps = prepare(x, y, fuse_w, fuse_b, se_w1, se_w2, bd_w, bd_b,
                          fc_w, fc_b, fm_w, fm_b, cv_w, cv_b)
    res = run_bass_kernel_spmd(nc, in_maps, core_ids=list(range(8)))
    LAST_RESULT = res
    outs = []
    for i in range(len(in_maps)):
        ot = np.asarray(res.results[i]["out"], np.float32)
        full = ot.reshape(NT, SB, 16, W).transpose(2, 0, 1, 3) \
                 .reshape(16, NT * SB, W)[:, :H, :]
        outs.append(full)
    return np.stack(outs)
